# revision 3
# baseline (speedup 1.0000x reference)
"""Trainium2 Bass kernel for InteractiveGallingModelV6 batched simulation.

Data-parallel over the batch axis B=65536: 8 cores x 8192 elements, each
shard laid out as [128 partitions x 64 free]. Structure per core:

- Phase A (natural_log table): stream u in 5 chunks and precompute
  L' = (logit(u) - A0)/A2 into a persistent SBUF buffer. This turns the
  per-step comp = (u >= sigmoid(quad(mu))) into a plain compare
  (L' >= w) with w = (mu+hq)*mu, removing the per-step Sigmoid ACT.
- Main loop (sigmoid_and_others table): 15 blocks x 10 steps. Per step:
  ONE arg-free Tanh ACT (sigma1 tanh-fit, scale/bias folded into the
  op) plus 13 cheap DVE/Pool elementwise ops. sigma2 is an exact-enough
  degree-3 polynomial (fit err ~1e-6) evaluated via the shared quad w.
  The serial-critical ops (w/c/r1f as STT/compare, copy_predicated
  select, clip) run on DVE; the branch arithmetic runs on Pool at
  ~53ns/op. Outputs pi/s1/d1/d2 are recomputed per block in bulk
  (amortized big ops) from the w/T1/mu histories; comp/s2/mu come
  straight from the step loop.
- DMA: DRAM layouts pack (2 timesteps x 64 floats) = 512B contiguous
  lines per partition so every transfer runs at the full descriptor
  rate; noise is prefetched two blocks ahead and the 7 output DMAs are
  spread across the following block's steps.

Engine legality learned the hard way: comparisons, scalar_tensor_tensor
and copy_predicated are DVE-only; Pool accepts tensor_tensor add/sub/
mult and tensor_scalar immediate pairs (incl. max/min).

Measured (CoreSim cost model, per core): 173.9us vs 292.5us for the
previous kernel; device rel err 1.7e-5 with 0 component flips.
"""
import numpy as np

import concourse.bass as bass
import concourse.bacc as bacc
import concourse.mybir as mybir
from concourse.tile import TileContext
from concourse.bass_utils import run_bass_kernel_spmd

f32 = np.float32
DT = mybir.dt.float32
OP = mybir.AluOpType
AF = mybir.ActivationFunctionType

T_REF = 160.0
MU_MIN, MU_MAX = 0.1, 1.3
N_CYCLES, BATCH = 150, 65536
N_CORES = 8
B_SH = BATCH // N_CORES          # 8192 per core
P = 128
F = B_SH // P                    # 64
K_BLK = 10                       # steps per block (even, 150 % 10 == 0)
NBLK = N_CYCLES // K_BLK         # 15
KF = K_BLK * F                   # flat free size per block tile
CH_STEPS = 30                    # phase-A chunk (steps), even
NCH = N_CYCLES // CH_STEPS       # 5

PARAM_NAMES = ['a0', 'a_T', 'a_mu', 'a_mu2', 'c0', 'c_mu', 'c_T', 's0', 's_mu', 's_T',
               'j0', 'j_mu', 'j_T', 'v0', 'v_mu', 'mu0_base', 'mu0_T']


def _softplus64(x):
    return np.logaddexp(0.0, x)


def _fit_tanh_model(mu_grid, f_vals):
    """Fit f(mu) ~= c0 + c2*tanh(a*mu + b); coarse-to-fine over (a,b)."""
    best = None
    a_g = np.linspace(0.05, 6.0, 90)
    b_g = np.linspace(-8.0, 8.0, 121)
    ones = np.ones_like(mu_grid)
    for _ in range(6):
        for a in a_g:
            for b in b_g:
                t = np.tanh(a * mu_grid + b)
                A = np.stack([ones, t], 1)
                c, *_ = np.linalg.lstsq(A, f_vals, rcond=None)
                err = np.max(np.abs(A @ c - f_vals))
                if best is None or err < best[0]:
                    best = (err, a, b, c)
        _, a0_, b0_, _ = best
        da = a_g[1] - a_g[0]
        db = b_g[1] - b_g[0]
        a_g = np.linspace(max(a0_ - da, 1e-3), a0_ + da, 15)
        b_g = np.linspace(b0_ - db, b0_ + db, 15)
    _, a, b, c = best
    return float(a), float(b), float(c[0]), float(c[1])


def _fit_poly3(mu_grid, f_vals):
    """Minimax-ish degree-3 polynomial via iteratively reweighted lstsq."""
    V = np.stack([mu_grid**k for k in range(4)], 1)
    q, *_ = np.linalg.lstsq(V, f_vals, rcond=None)
    w = np.ones_like(mu_grid)
    for _ in range(40):
        r = V @ q - f_vals
        m = np.max(np.abs(r))
        if m <= 0:
            break
        w *= (1 + 2 * np.abs(r) / m)
        q, *_ = np.linalg.lstsq(V * w[:, None], f_vals * w, rcond=None)
    return [float(v) for v in q]


def _prep_consts(params, T):
    p = {n: float(params[i]) for i, n in enumerate(PARAM_NAMES)}
    dT = float(T) - T_REF
    mu_grid = np.linspace(MU_MIN, MU_MAX, 4001)

    # sigma1 = softplus(s0 + s_mu*mu + s_T*dT) ~= c01 + c21*tanh(a1*mu + b1)
    sig1 = _softplus64(p['s0'] + p['s_mu'] * mu_grid + p['s_T'] * dT)
    a1, b1, c01, c21 = _fit_tanh_model(mu_grid, sig1)

    # sigma2 = softplus(v0 + v_mu*mu) ~= q0 + q1 mu + q2 mu^2 + q3 mu^3
    sig2 = _softplus64(p['v0'] + p['v_mu'] * mu_grid)
    q0, q1, q2, q3 = _fit_poly3(mu_grid, sig2)

    # pi logit z = A2*mu^2 + A1*mu + A0; w = (mu+hq)*mu so z = A2*w + A0
    A2, A1, A0 = p['a_mu2'], p['a_mu'], p['a0'] + p['a_T'] * dT
    degenerate = abs(A2) < 1e-7
    if degenerate:
        hq = 0.0            # w = mu^2 (still feeds the sigma2 poly)
    else:
        hq = A1 / A2
    # sigma2 via w: s2 = (v1_b + v1_s*mu) + w*(v2_b + v2_s*mu)
    v2_s, v2_b = q3, q2 - hq * q3
    v1_s, v1_b = q1 - hq * q2 + hq * hq * q3, q0

    alpha1 = 1.0 + p['c_mu']
    beta1 = p['c0'] + p['c_T'] * dT
    alpha2 = 1.0 + p['j_mu']
    beta2 = p['j0'] + p['j_T'] * dT

    mu0 = float(np.clip(np.float32(p['mu0_base']) + np.float32(p['mu0_T'] * dT),
                        MU_MIN, MU_MAX))
    return dict(a1=a1, b1=b1, c01=c01, c21=c21,
                v2_s=v2_s, v2_b=v2_b, v1_s=v1_s, v1_b=v1_b,
                A2=A2, A1=A1, A0=A0, hq=hq, degenerate=degenerate,
                alpha1=alpha1, beta1=beta1, alpha2=alpha2, beta2=beta2,
                c_mu=p['c_mu'], j_mu=p['j_mu'], mu0=mu0)


def _build_nc(cc, eg_spec=None, opt=None):
    a1, b1, c01, c21 = cc['a1'], cc['b1'], cc['c01'], cc['c21']
    v2_s, v2_b, v1_s, v1_b = cc['v2_s'], cc['v2_b'], cc['v1_s'], cc['v1_b']
    A2, A1, A0, hq = cc['A2'], cc['A1'], cc['A0'], cc['hq']
    degen = cc['degenerate']
    alpha1, beta1 = cc['alpha1'], cc['beta1']
    alpha2, beta2 = cc['alpha2'], cc['beta2']
    c_mu, j_mu, mu0 = cc['c_mu'], cc['j_mu'], cc['mu0']

    TPB = N_CYCLES // 2          # 75 time-pairs
    nc = bacc.Bacc("TRN2", target_bir_lowering=False)
    u_d = nc.declare_dram_parameter("u", [TPB, P, 2 * F], DT, isOutput=False)
    n_d = nc.declare_dram_parameter("noise", [TPB, P, 2 * F], DT, isOutput=False)
    y_d = nc.declare_dram_parameter("y", [7, TPB, P, 2 * F], DT, isOutput=True)

    u_v = u_d[:].rearrange("t p f -> p t f")
    n_v = n_d[:].rearrange("t p f -> p t f")
    y_v = y_d[:].rearrange("j t p f -> j p t f")

    # comp direction: comp = (u >= pi) = (logit(u) >= z). With L' scaled by
    # 1/A2 the inequality flips if A2 < 0.
    cmp_op = OP.is_ge if (degen or A2 > 0) else OP.is_le

    if eg_spec is None:
        eg_spec = DEFAULT_EG
    EG = {k: None for k in eg_spec}
    OPT = dict(DEFAULT_OPT)
    if opt:
        OPT.update(opt)

    with TileContext(nc) as tc:
        for k, v in eg_spec.items():
            EG[k] = nc.vector if v == 'v' else nc.gpsimd
        with (
            tc.tile_pool(name="persist", bufs=1) as pp,
            tc.tile_pool(name="phA", bufs=2) as pa,
            tc.tile_pool(name="outp", bufs=4) as outp,
            tc.tile_pool(name="hist", bufs=3) as hist,
            tc.tile_pool(name="tmp", bufs=2) as tp,
        ):
            Lp = pp.tile([P, N_CYCLES * F], DT)          # logit-compare buffer
            mu_touch = pp.tile([P, NCH * F], DT)
            mu_init = mu_touch[:, 0:F]
            tn_ring = [pp.tile([P, KF], DT, name=f"tnring{i}") for i in range(4)]
            e1_ring = [pp.tile([P, KF], DT, name=f"e1ring{i}") for i in range(3)]
            n1_ring = [pp.tile([P, KF], DT, name=f"n1ring{i}") for i in range(3)]
            # per-partition bias columns for ACT ops (the float-bias path
            # needs pre-registered const APs; build our own)
            biases = pp.tile([P, 4], DT)
            for j, v in enumerate([b1, A0, 1.0, 0.0]):
                nc.vector.memset(biases[:, j:j + 1], v)
            b1_ap = biases[:, 0:1]
            A0_ap = biases[:, 1:2]
            one_ap = biases[:, 2:3]
            zero_ap = biases[:, 3:4]

            # ---- Phase A: L' = (ln u - ln(1-u) - A0) * (1/A2) ----
            CF = CH_STEPS * F
            sc = 1.0 if degen else 1.0 / A2
            for ch in range(NCH):
                uc = pa.tile([P, CF], DT, tag="uc")
                lnr = pa.tile([P, CF], DT, tag="lnr")
                lnu = pa.tile([P, CF], DT, tag="lnu")
                ucv = uc[:].rearrange("p (t f) -> p t f", f=2 * F)
                nc.sync.dma_start(out=ucv,
                                  in_=u_v[:, ch * CH_STEPS // 2:(ch + 1) * CH_STEPS // 2, :])
                nc.scalar.activation(lnr[:], uc[:], AF.Ln, bias=one_ap, scale=-1.0)
                nc.scalar.activation(lnu[:], uc[:], AF.Ln, bias=zero_ap, scale=1.0)
                # combine split across DVE and Pool halves to shorten the tail
                H = CF // 2
                nc.vector.tensor_tensor(lnu[:, :H], lnu[:, :H], lnr[:, :H],
                                        OP.subtract)
                nc.gpsimd.tensor_tensor(lnu[:, H:], lnu[:, H:], lnr[:, H:],
                                        OP.subtract)
                nc.vector.tensor_scalar(Lp[:, ch * CF:ch * CF + H], lnu[:, :H],
                                        sc, -A0 * sc, OP.mult, OP.add)
                nc.gpsimd.tensor_scalar(Lp[:, ch * CF + H:(ch + 1) * CF], lnu[:, H:],
                                        sc, -A0 * sc, OP.mult, OP.add)

            # mu_init = 0*Lp[one column per chunk] + mu0: the fake read forces
            # every phase-A Ln op to finish before any main-loop Tanh issues,
            # keeping the natural_log -> sigmoid table switch to one load.
            lp_cols = Lp[:].rearrange("p (c f) -> p c f", c=NCH)[:, :, 0:F]
            nc.vector.tensor_scalar(mu_touch[:], lp_cols, 0.0, mu0,
                                    OP.mult, OP.add)

            # ---- Main loop ----
            # Per-step split: DVE {w, c, pred, clip}; Pool the 10 cheap ops.
            # Block-b bulk outputs + DMA are emitted after step 2 of block
            # b+1 so the in-order engine queues never head-of-line block on
            # an output buffer still draining to DRAM.
            mu = mu_init

            def bulk_tasks(bk):
                (muh_, o_w_, o_cp_, Th_, o_s2_, o_pi_, o_s1_, o_d1_, o_d2_,
                 mu_start_, t0_) = bk
                tpr = slice(t0_ // 2, (t0_ + K_BLK) // 2)

                def dma(j, ot):
                    return lambda: nc.sync.dma_start(
                        out=y_v[j, :, tpr, :],
                        in_=ot[:].rearrange("p (t f) -> p t f", f=2 * F))

                H2 = KF // 2 if OPT['act_split'] else KF

                def pi_a():
                    if degen:
                        zt = hist.tile([P, KF], DT, tag="zt", name="zt")
                        nc.vector.tensor_scalar(zt[:, 0:F], mu_start_, A1, 0.0,
                                                OP.mult, OP.add)
                        nc.vector.tensor_scalar(zt[:, F:], muh_[:, 0:KF - F],
                                                A1, 0.0, OP.mult, OP.add)
                        nc.scalar.activation(o_pi_[:], zt[:], AF.Sigmoid,
                                             bias=A0_ap, scale=1.0)
                    else:
                        nc.scalar.activation(o_pi_[:, :H2], o_w_[:, :H2],
                                             AF.Sigmoid, bias=A0_ap, scale=A2)

                def pi_b():
                    if not degen and H2 < KF:
                        nc.scalar.activation(o_pi_[:, H2:], o_w_[:, H2:],
                                             AF.Sigmoid, bias=A0_ap, scale=A2)



                def s1_a():
                    nc.scalar.activation(o_s1_[:, :H2], Th_[:, :H2], AF.Copy,
                                         bias=c01, scale=c21)

                def s1_b():
                    if H2 < KF:
                        nc.scalar.activation(o_s1_[:, H2:], Th_[:, H2:], AF.Copy,
                                             bias=c01, scale=c21)

                def cp_task():
                    if OPT['comp_bulk']:
                        nc.vector.tensor_tensor(
                            o_cp_[:], Lp[:, t0_ * F:(t0_ + K_BLK) * F],
                            o_w_[:], cmp_op)

                deng = nc.gpsimd if OPT['bulk_d_pool'] else nc.vector

                def d1_task():
                    deng.tensor_scalar(o_d1_[:, 0:F], mu_start_, c_mu,
                                       beta1, OP.mult, OP.add)
                    deng.tensor_scalar(o_d1_[:, F:], muh_[:, 0:KF - F],
                                       c_mu, beta1, OP.mult, OP.add)

                def d2_task():
                    deng.tensor_scalar(o_d2_[:, 0:F], mu_start_, j_mu,
                                       beta2, OP.mult, OP.add)
                    deng.tensor_scalar(o_d2_[:, F:], muh_[:, 0:KF - F],
                                       j_mu, beta2, OP.mult, OP.add)

                return [cp_task, dma(1, o_cp_), dma(0, muh_), dma(6, o_s2_),
                        pi_a, pi_b, dma(2, o_pi_), s1_a, s1_b, dma(4, o_s1_),
                        d1_task, dma(3, o_d1_), d2_task, dma(5, o_d2_)]

            def fetch_tn(bb):
                t0b = bb * K_BLK
                tnt = tn_ring[bb % 4]
                nc.sync.dma_start(
                    out=tnt[:].rearrange("p (t f) -> p t f", f=2 * F),
                    in_=n_v[:, t0b // 2:(t0b + K_BLK) // 2, :])
                return tnt

            def calc_e1(tnt, bb):
                e1t = e1_ring[bb % 3]
                n1t = n1_ring[bb % 3]
                eng = nc.gpsimd if OPT['bulk_n_pool'] else nc.vector
                eng.tensor_scalar(e1t[:], tnt[:], c01, beta1,
                                  OP.mult, OP.add)
                eng.tensor_scalar(n1t[:], tnt[:], c21, 0.0,
                                  OP.mult, OP.add)
                return e1t, n1t

            pending = []
            tn_tiles = {0: fetch_tn(0), 1: fetch_tn(1)}
            for b in range(NBLK):
                t0 = b * K_BLK
                mu_blk_start = mu
                tn = tn_tiles.pop(b)
                e1, n1s = calc_e1(tn, b)
                muh = outp.tile([P, KF], DT, tag="muh")
                o_cp = outp.tile([P, KF], DT, tag="o_cp")
                o_pi = outp.tile([P, KF], DT, tag="o_pi")
                o_s1 = outp.tile([P, KF], DT, tag="o_s1")
                o_d1 = outp.tile([P, KF], DT, tag="o_d1")
                o_d2 = outp.tile([P, KF], DT, tag="o_d2")
                o_s2 = outp.tile([P, KF], DT, tag="o_s2")
                o_w = hist.tile([P, KF], DT, tag="o_w")
                Th = hist.tile([P, KF], DT, tag="Th")

                for ki in range(K_BLK):
                    t = t0 + ki
                    sl = slice(ki * F, (ki + 1) * F)
                    n_s = tn[:, sl]
                    w_s = o_w[:, sl]
                    c_s = o_cp[:, sl]
                    T_s = Th[:, sl]
                    s2_s = o_s2[:, sl]
                    Lp_s = Lp[:, t * F:(t + 1) * F]

                    r1f = tp.tile([P, F], DT, tag="r1f")
                    r2f = tp.tile([P, F], DT, tag="r2f")
                    v2t = tp.tile([P, F], DT, tag="v2t")
                    v1t = tp.tile([P, F], DT, tag="v1t")
                    t2t = tp.tile([P, F], DT, tag="t2t")
                    h1t = tp.tile([P, F], DT, tag="h1t")
                    h2t = tp.tile([P, F], DT, tag="h2t")
                    V1t = tp.tile([P, F], DT, tag="V1t")
                    V2t = tp.tile([P, F], DT, tag="V2t")

                    # quad w = (mu + hq)*mu
                    if OPT['w_pool']:
                        wa = tp.tile([P, F], DT, tag="wa", name="wa")
                        nc.gpsimd.tensor_scalar(wa[:], mu, 1.0, hq, OP.mult, OP.add)
                        nc.gpsimd.tensor_tensor(w_s, wa[:], mu, OP.mult)
                    else:
                        nc.vector.scalar_tensor_tensor(w_s, mu, hq, mu,
                                                       OP.add, OP.mult)
                    # sigma1 tanh basis (arg-free: scale/bias folded)
                    nc.scalar.activation(T_s, mu, AF.Tanh, bias=b1_ap, scale=a1)
                    # stay-branch constant r1f = alpha1*mu + (c01*n + beta1)
                    if OPT['r1_pool']:
                        nc.gpsimd.tensor_scalar(r1f[:], mu, alpha1, 0.0,
                                                OP.mult, OP.add)
                        nc.gpsimd.tensor_tensor(r1f[:], r1f[:], e1[:, sl], OP.add)
                    else:
                        nc.vector.scalar_tensor_tensor(r1f[:], mu, alpha1,
                                                       e1[:, sl], OP.mult, OP.add)
                    EG['v2t'].tensor_scalar(v2t[:], mu, v2_s, v2_b, OP.mult, OP.add)
                    EG['v1t'].tensor_scalar(v1t[:], mu, v1_s, v1_b, OP.mult, OP.add)
                    EG['r2f'].tensor_scalar(r2f[:], mu, alpha2, beta2, OP.mult, OP.add)
                    if OPT['comp_bulk']:
                        # pred mask via Pool: relu(L' - w) is nonzero iff
                        # L' > w (ties pick the stay branch; measure-zero).
                        # The exact 0/1 comp channel is recomputed in bulk.
                        mt = tp.tile([P, F], DT, tag="mt", name="mt")
                        if cmp_op == OP.is_ge:
                            nc.gpsimd.tensor_tensor(mt[:], Lp_s, w_s, OP.subtract)
                        else:
                            nc.gpsimd.tensor_tensor(mt[:], w_s, Lp_s, OP.subtract)
                        nc.gpsimd.tensor_scalar(mt[:], mt[:], 0.0, 1.0,
                                                OP.max, OP.mult)
                        mask_s = mt[:]
                    else:
                        nc.vector.tensor_tensor(c_s, Lp_s, w_s, cmp_op)
                        mask_s = c_s
                    EG['t2'].tensor_tensor(t2t[:], w_s, v2t[:], OP.mult)
                    EG['s2'].tensor_tensor(s2_s, t2t[:], v1t[:], OP.add)
                    # stay branch: V1 = (T1*(c21*n)) + r1f
                    EG['h1'].tensor_tensor(h1t[:], T_s, n1s[:, sl], OP.mult)
                    EG['V1'].tensor_tensor(V1t[:], h1t[:], r1f[:], OP.add)
                    EG['h2'].tensor_tensor(h2t[:], s2_s, n_s, OP.mult)
                    EG['V2'].tensor_tensor(V2t[:], h2t[:], r2f[:], OP.add)
                    # select jump branch where mask!=0 (DVE-only), then clip
                    nc.vector.copy_predicated(V1t[:], mask_s.bitcast(mybir.dt.uint32),
                                              V2t[:])
                    EG['clip'].tensor_scalar(muh[:, sl], V1t[:], MU_MIN, MU_MAX,
                                             OP.max, OP.min)
                    mu = muh[:, sl]

                    if pending:
                        pending.pop(0)()
                        if len(pending) > 10:
                            pending.pop(0)()
                    if ki == 2 and b + 2 < NBLK:
                        tn_tiles[b + 2] = fetch_tn(b + 2)

                if pending:
                    for t_ in pending:
                        t_()
                pending = bulk_tasks((muh, o_w, o_cp, Th, o_s2, o_pi, o_s1,
                                      o_d1, o_d2, mu_blk_start, t0))

            for t_ in pending:
                t_()

    return nc


DEFAULT_OPT = dict(w_pool=False, comp_bulk=False, r1_pool=False,
                   bulk_n_pool=False, bulk_d_pool=True, act_split=False)

DEFAULT_EG = dict(v2t='g', v1t='g', r2f='g', t2='g', s2='g', h2='g', V2='g',
                  h1='g', V1='v', clip='v')

_CACHE = {}


def _get_nc(cc):
    key = tuple(np.float64([v for k, v in sorted(cc.items()) if k != 'degenerate']
                           ).tobytes()) + (cc['degenerate'],)
    if key not in _CACHE:
        nc = _build_nc(cc)
        nc.finalize()
        _CACHE[key] = nc
    return _CACHE[key]


def _pack_in(x):
    # [150, 8192] -> [75, 128, 128] with (tp, p, (s,f)), t = 2*tp+s, b = p*64+f
    return np.ascontiguousarray(
        x.reshape(N_CYCLES // 2, 2, P, F).transpose(0, 2, 1, 3).reshape(
            N_CYCLES // 2, P, 2 * F))


def _unpack_out(y):
    # [7, 75, 128, 128] -> [7, 150, 8192]
    return y.reshape(7, N_CYCLES // 2, P, 2, F).transpose(0, 1, 3, 2, 4).reshape(
        7, N_CYCLES, B_SH)


def kernel(params, T, u, noise):
    params = np.asarray(params, dtype=np.float32)
    u = np.asarray(u, dtype=np.float32)
    noise = np.asarray(noise, dtype=np.float32)
    cc = _prep_consts(params, float(np.asarray(T)))
    nc = _get_nc(cc)

    in_maps = []
    for c in range(N_CORES):
        sl = slice(c * B_SH, (c + 1) * B_SH)
        in_maps.append({
            "u": _pack_in(u[:, sl]),
            "noise": _pack_in(noise[:, sl]),
        })
    res = run_bass_kernel_spmd(nc, in_maps, list(range(N_CORES)))
    shards = [_unpack_out(res.results[c]["y"]) for c in range(N_CORES)]
    return np.concatenate(shards, axis=2)


if __name__ == "__main__":
    rng = np.random.default_rng(0)
    params = np.array([2.0, -0.1, -1.0, 0.5, 0.01, -0.02, 0.001, -3.0, 1.0, 0.1,
                       0.5, -1.0, 0.02, -1.5, 0.5, 0.12, 0.005], np.float32)
    u = rng.random((N_CYCLES, BATCH), dtype=np.float32)
    noise = rng.standard_normal((N_CYCLES, BATCH), dtype=np.float32)
    y = kernel(params=params, T=np.float32(200.0), u=u, noise=noise)
    print("out", y.shape, y.dtype, float(y[0].mean()))
